# revision 37
# baseline (speedup 1.0000x reference)
"""NT-Xent (SimCLR) contrastive loss on 8 Trainium2 NeuronCores.

Math: with x = row-normalized representation [8192, 256], tau = 0.5,
  sim = x @ x.T
  loss = (1/8192) * sum_i [ ln(sum_{j != i} exp(2 sim[i,j])) - 2 sim[i, pos(i)] ]
where pos(i) = (i + 4096) mod 8192.

Split of work:
  Host (O(N*D), numpy): row-normalize, cast to bf16, per-core row-rolled
  and transposed copies, positive-pair dots, final combine/ln/mean.
  Device (O(N^2*D)): the similarity matrix and row/column sums of exp(2 sim).

Symmetry: sim is symmetric, so only 5/8 of it is computed. Each core gets
xT rolled so its 1024 slab rows sit at columns 0..1023 of xT, and computes
sim[0:1024, 0:5120] (column blocks b=0..4; b=4 is computed by both members
of a (c, c+4) pair). Row sums of exp(2 sim) cover blocks 0..4; the missing
blocks 5..7 are transposes of blocks 1..3 of three other cores, recovered
from COLUMN sums of exp over blocks 1..3, computed on-device with
ones-stationary matmuls accumulating in two PSUM banks across the 8
row-tiles. The host combines row + column partials into full row sums.

Device kernel (SPMD, identical program on all 8 cores):
  1. The host supplies xT [256, 8192] bf16 (normalized, rolled,
     transposed), so SBUF keys load with 5 plain chunk-aligned DMAs
     (both k-halves per chunk in one 3D-AP DMA). The slab tile and the
     first sweep's tile come first.
  2. Four chunk-sweeps over columns, widths {512,1536,1536,1536}, the
     cheap 512 sweep first (it depends on two small DMAs and its dense
     matmul stream warms the HAM clock gate). Per row-tile m: k-outer
     bf16 matmuls (N=512) into a [128,1536] PSUM tile (2 bufs, 6 banks),
     then one scalar Exp (scale=2) per chunk -> bf16 SBUF. The exp
     activation table is preloaded during the DMA fill.
  3. Row sums: DVE reduce_sum for the three 1536 sweeps (the DVE is
     otherwise idle), accum_out on the 512 sweep -> rs [128, 32].
  4. Column sums: six ones-stationary matmuls per m over exp'd
     [128,512] slices of blocks 1..3, accumulated over m in two PSUM
     banks at partitions {0,32,64}, emitted one m behind in the last
     sweep to keep the PE stream dense. DVE copies them out at the end.
"""

import numpy as np
import ml_dtypes

import concourse.bacc as bacc
import concourse.tile as tile
from concourse import mybir
from concourse.bass_utils import run_bass_kernel_spmd

N2 = 8192            # total rows (2N)
D = 256              # feature dim
NCORES = 8
ROWS = N2 // NCORES  # 1024 slab rows per core
N = N2 // 2          # positive-pair offset
P = 128              # SBUF partitions
KC = D // P          # 2 contraction chunks of 128
MT = ROWS // P       # 8 slab row-tiles
COLS = 5120          # columns computed per slab row-tile (blocks 0..4)
# xT SBUF tiles: (col start, width); tile 0 holds the slab columns
XT = [(0, 1024), (4608, 512), (1024, 512), (1536, 1536), (3072, 1536)]
# chunk sweeps in execution order: (col start, width, chunk id)
SWEEPS = [(4608, 512, 3), (0, 1536, 0), (1536, 1536, 1), (3072, 1536, 2)]
NCH = 4
ACC_CH = 3           # the only chunk using scalar accum_out (rest: DVE)
# column-sum slices covering rolled cols [1024, 4096) as (chunk, offset)
RED = [(0, 1024), (1, 0), (1, 512), (1, 1024), (2, 0), (2, 512)]

F32 = mybir.dt.float32
BF16 = mybir.dt.bfloat16
F8 = mybir.dt.float8e4
AF = mybir.ActivationFunctionType
QS = 16.0            # fp8 quantization scale for xT (values ~N(0, 1/16))
ESC = 2.0 / (QS * QS)  # exp scale: sim = psum / QS^2, arg = 2*sim


def _xt_tile(col):
    for t, (s, w) in enumerate(XT):
        if s <= col < s + w:
            return t, col - s
    raise ValueError(col)


def _build_kernel(tc: tile.TileContext, rs_out, cols_out, xth):
    nc = tc.nc
    with (
        tc.tile_pool(name="xt", bufs=1) as xtp,
        tc.tile_pool(name="acc", bufs=1) as accp,
        tc.tile_pool(name="esc0", bufs=MT) as escp0,
        tc.tile_pool(name="esc1", bufs=MT) as escp1,
        tc.tile_pool(name="esc2", bufs=MT) as escp2,
        tc.tile_pool(name="esc3", bufs=2) as escp3,
        tc.tile_pool(name="ps", bufs=2, space="PSUM") as psp,
        tc.tile_pool(name="red", bufs=1, space="PSUM") as redp,
    ):
        # each xT tile holds both k-halves: [:, k*w : k*w+w]
        xts = [xtp.tile([P, KC * w], F8, tag=f"xt{t}", name=f"xt{t}")
               for t, (s, w) in enumerate(XT)]
        rs = accp.tile([P, NCH * MT], F32, tag="rs", name="rs")
        ones = accp.tile([P, 1], BF16, tag="ones", name="ones")
        dume = accp.tile([P, 1], BF16, tag="dume", name="dume")
        colsb = accp.tile([P, 1024], F32, tag="colsb", name="colsb")
        red = [redp.tile([P, 512], F32, tag=f"red{t}", name=f"red{t}")
               for t in range(2)]
        nc.vector.memset(ones, 1.0)
        # pull the exp ACT_TABLE_LOAD into the DMA fill window
        nc.scalar.activation(dume, ones, AF.Exp, scale=1.0)

        # mini copy of the m=0 stationary columns so the very first matmuls
        # gate on a 32KB DMA instead of the full 256KB slab tile
        xa0 = xtp.tile([P, KC * P], F8, tag="xa0", name="xa0")
        # scalar HWDGE ring: lands in parallel with the sync ring's first DMA
        nc.scalar.dma_start(
            out=xa0, in_=xth[:, 0:P].rearrange("(k p) c -> p k c", k=KC))

        # plain DMAs of host-pretransposed keys; one 3D-AP DMA per tile
        # loads both k-halves (dest free block k*w..k*w+w <- xT rows
        # k*128..k*128+128, cols s..s+w); first sweep's moving tile first
        for t in (1, 0, 2, 3, 4):
            s, w = XT[t]
            nc.sync.dma_start(
                out=xts[t],
                in_=xth[:, s:s + w].rearrange("(k p) c -> p k c", k=KC))

        # 3D views [p, k, c] for DoubleRow matmuls (both k-tiles in one MM)
        xt3 = [xts[t].rearrange("p (k c) -> p k c", k=KC)
               for t in range(len(XT))]
        xa03 = xa0.rearrange("p (k c) -> p k c", k=KC)

        def mov(col):  # moving operand [128, 2, 512] for global column col
            t, off = _xt_tile(col)
            return xt3[t][:, :, off:off + 512]

        escs = {}
        pools = {0: escp0, 1: escp1, 2: escp2, 3: escp3}

        def red_mms(m, which):
            for i, (rc, off) in enumerate(RED):
                if rc not in which:
                    continue
                t, bp = i % 2, 32 * (i // 2)
                nc.tensor.matmul(
                    red[t][bp:bp + 1, :],
                    ones,
                    escs[(m, rc)][:, off:off + 512],
                    start=(m == 0), stop=(m == MT - 1),
                    skip_group_check=True)

        last = SWEEPS[-1][2]
        for cs, w, c in SWEEPS:
            for m in range(MT):
                ps = psp.tile([P, 1536], F32, tag="ps", name="ps")
                stat = xa03 if m == 0 else xt3[0][:, :, m * P:(m + 1) * P]
                for half in range(w // 512):
                    nc.tensor.matmul(
                        ps[:, half * 512:(half + 1) * 512],
                        stat,
                        mov(cs + half * 512),
                        start=True, stop=True,
                        perf_mode=mybir.MatmulPerfMode.DoubleRow)
                ridx = m * NCH + c
                if c == ACC_CH:
                    # b4's exp values feed nothing but the row sum: keep
                    # them in PSUM (faster ScalarE port, no SBUF tile)
                    nc.scalar.activation(
                        ps[:, w:2 * w], ps[:, :w], AF.Exp, scale=ESC,
                        accum_out=rs[:, ridx:ridx + 1])
                    continue
                esc = pools[c].tile([P, w], BF16, tag="esc", name="esc")
                escs[(m, c)] = esc
                if c == last and m >= MT - 2:
                    nc.scalar.activation(
                        esc, ps[:, :w], AF.Exp, scale=ESC,
                        accum_out=rs[:, ridx:ridx + 1])
                else:  # row sum on DVE, off the scalar engine
                    nc.scalar.activation(esc, ps[:, :w], AF.Exp, scale=ESC)
                    nc.vector.reduce_sum(rs[:, ridx:ridx + 1], esc,
                                         axis=mybir.AxisListType.X)
                # column-sum matmuls ride the last sweep: slices reading
                # the finished c0/c1 sweeps go with the current m; the two
                # slices reading this sweep's own exp output go one behind
                if c == last:
                    red_mms(m, (0, 1))
                    if m > 0:
                        red_mms(m - 1, (2,))
        red_mms(MT - 1, (2,))

        for t in range(2):
            nc.vector.tensor_copy(colsb[:, t * 512:(t + 1) * 512], red[t])
        nc.sync.dma_start(out=rs_out, in_=rs)
        # only partitions {0,32,64} carry column sums: ship 12KB, not 512KB
        nc.sync.dma_start(out=cols_out, in_=colsb[0:65:32, :])


def build_nc():
    nc = bacc.Bacc("TRN2", target_bir_lowering=False, debug=False,
                   num_devices=NCORES)
    xth = nc.dram_tensor("xt", [D, N2], F8, kind="ExternalInput").ap()
    rs_out = nc.dram_tensor("rs", [P, NCH * MT], F32,
                            kind="ExternalOutput").ap()
    cols_out = nc.dram_tensor("cols", [3, 1024], F32,
                              kind="ExternalOutput").ap()
    with tile.TileContext(nc) as tc:
        _build_kernel(tc, rs_out, cols_out, xth)
    nc.compile()
    return nc


_NC = None
LAST_RESULTS = None


def _make_in_maps(xq: np.ndarray):
    in_maps = []
    for c in range(NCORES):
        xr = np.roll(xq, -c * ROWS, axis=0)
        in_maps.append({"xt": np.ascontiguousarray(xr.T)})
    return in_maps


def kernel(representation: np.ndarray, **run_kwargs) -> np.ndarray:
    global _NC, LAST_RESULTS
    rep = np.asarray(representation, dtype=np.float32)
    assert rep.shape == (N2, D)

    # host prep: normalize (f32, matching torch CosineSimilarity eps),
    # scale by QS and quantize to fp8e4m3 for DoubleRow matmuls
    norms = np.maximum(np.sqrt((rep.astype(np.float64) ** 2).sum(axis=1)),
                       1e-8)
    xn = (rep / norms[:, None]).astype(np.float32)
    xq = (xn * QS).astype(ml_dtypes.float8_e4m3fn)

    if _NC is None:
        _NC = build_nc()
    res = run_bass_kernel_spmd(_NC, _make_in_maps(xq),
                               core_ids=list(range(NCORES)), **run_kwargs)
    LAST_RESULTS = res

    # combine row partials (rolled cols 0..5120) and column partials
    # (rolled cols 1024..4096, blocks b=1..3) into full row sums S
    S = np.zeros(N2, dtype=np.float64)
    for c, r in enumerate(res.results):
        rs = r["rs"].astype(np.float64).reshape(P, MT, NCH)  # [p, m, ch]
        own = rs.sum(axis=2).T.reshape(ROWS)                 # row m*128+p
        S[c * ROWS:(c + 1) * ROWS] += own
        cols = r["cols"].astype(np.float64)                  # [3, 1024]
        for i in range(6):
            colsum = cols[i // 2, (i % 2) * 512:(i % 2) * 512 + 512]
            g0 = (c * ROWS + 1024 + 512 * i) % N2
            S[g0:g0 + 512] += colsum

    # host tail: remove diagonal (as the device computed it, from fp8
    # inputs), add positive terms, final log/mean
    xb = xq.astype(np.float64) / QS
    ssb = (xb * xb).sum(axis=1)                  # device's sim[i,i]
    denom = S - np.exp(2.0 * ssb)
    xn64 = xn.astype(np.float64)
    pos = (xn64 * np.roll(xn64, -N, axis=0)).sum(axis=1)
    loss = (np.log(denom) - 2.0 * pos).mean()
    return np.asarray(np.float32(loss))


# revision 38
# speedup vs baseline: 1.0526x; 1.0526x over previous
"""NT-Xent (SimCLR) contrastive loss on 8 Trainium2 NeuronCores.

Math: with x = row-normalized representation [8192, 256], tau = 0.5,
  sim = x @ x.T
  loss = (1/8192) * sum_i [ ln(sum_{j != i} exp(2 sim[i,j])) - 2 sim[i, pos(i)] ]
where pos(i) = (i + 4096) mod 8192.

Split of work:
  Host (O(N*D), numpy): row-normalize, cast to bf16, per-core row-rolled
  and transposed copies, positive-pair dots, final combine/ln/mean.
  Device (O(N^2*D)): the similarity matrix and row/column sums of exp(2 sim).

Symmetry: sim is symmetric, so only 5/8 of it is computed. Each core gets
xT rolled so its 1024 slab rows sit at columns 0..1023 of xT, and computes
sim[0:1024, 0:5120] (column blocks b=0..4; b=4 is computed by both members
of a (c, c+4) pair). Row sums of exp(2 sim) cover blocks 0..4; the missing
blocks 5..7 are transposes of blocks 1..3 of three other cores, recovered
from COLUMN sums of exp over blocks 1..3, computed on-device with
ones-stationary matmuls accumulating in two PSUM banks across the 8
row-tiles. The host combines row + column partials into full row sums.

Device kernel (SPMD, identical program on all 8 cores):
  1. The host supplies xT [256, 8192] bf16 (normalized, rolled,
     transposed), so SBUF keys load with 5 plain chunk-aligned DMAs
     (both k-halves per chunk in one 3D-AP DMA). The slab tile and the
     first sweep's tile come first.
  2. Four chunk-sweeps over columns, widths {512,1536,1536,1536}, the
     cheap 512 sweep first (it depends on two small DMAs and its dense
     matmul stream warms the HAM clock gate). Per row-tile m: k-outer
     bf16 matmuls (N=512) into a [128,1536] PSUM tile (2 bufs, 6 banks),
     then one scalar Exp (scale=2) per chunk -> bf16 SBUF. The exp
     activation table is preloaded during the DMA fill.
  3. Row sums: DVE reduce_sum for the three 1536 sweeps (the DVE is
     otherwise idle), accum_out on the 512 sweep -> rs [128, 32].
  4. Column sums: six ones-stationary matmuls per m over exp'd
     [128,512] slices of blocks 1..3, accumulated over m in two PSUM
     banks at partitions {0,32,64}, emitted one m behind in the last
     sweep to keep the PE stream dense. DVE copies them out at the end.
"""

import numpy as np
import ml_dtypes

import concourse.bacc as bacc
import concourse.tile as tile
from concourse import mybir
from concourse.bass_utils import run_bass_kernel_spmd

N2 = 8192            # total rows (2N)
D = 256              # feature dim
NCORES = 8
ROWS = N2 // NCORES  # 1024 slab rows per core
N = N2 // 2          # positive-pair offset
P = 128              # SBUF partitions
KC = D // P          # 2 contraction chunks of 128
MT = ROWS // P       # 8 slab row-tiles
COLS = 5120          # columns computed per slab row-tile (blocks 0..4)
# xT SBUF tiles: (col start, width); tile 0 holds the slab columns
XT = [(0, 1024), (4608, 512), (1024, 512), (1536, 1536), (3072, 1536)]
# chunk sweeps in execution order: (col start, width, chunk id)
SWEEPS = [(4608, 512, 3), (0, 1536, 0), (1536, 1536, 1), (3072, 1536, 2)]
NCH = 4
ACC_CH = 3           # the only chunk using scalar accum_out (rest: DVE)
# column-sum slices covering rolled cols [1024, 4096) as (chunk, offset)
RED = [(0, 1024), (1, 0), (1, 512), (1, 1024), (2, 0), (2, 512)]

F32 = mybir.dt.float32
BF16 = mybir.dt.bfloat16
F8 = mybir.dt.float8e4
AF = mybir.ActivationFunctionType
QS = 16.0            # fp8 quantization scale for xT (values ~N(0, 1/16))
ESC = 2.0 / (QS * QS)  # exp scale: sim = psum / QS^2, arg = 2*sim


def _xt_tile(col):
    for t, (s, w) in enumerate(XT):
        if s <= col < s + w:
            return t, col - s
    raise ValueError(col)


def _build_kernel(tc: tile.TileContext, rs_out, cols_out, xth):
    nc = tc.nc
    with (
        tc.tile_pool(name="xt", bufs=1) as xtp,
        tc.tile_pool(name="acc", bufs=1) as accp,
        tc.tile_pool(name="esc0", bufs=MT) as escp0,
        tc.tile_pool(name="esc1", bufs=MT) as escp1,
        tc.tile_pool(name="esc2", bufs=MT) as escp2,
        tc.tile_pool(name="esc3", bufs=2) as escp3,
        tc.tile_pool(name="ps", bufs=2, space="PSUM") as psp,
        tc.tile_pool(name="red", bufs=1, space="PSUM") as redp,
    ):
        # each xT tile holds both k-halves: [:, k*w : k*w+w]
        xts = [xtp.tile([P, KC * w], F8, tag=f"xt{t}", name=f"xt{t}")
               for t, (s, w) in enumerate(XT)]
        rs = accp.tile([P, NCH * MT], F32, tag="rs", name="rs")
        ones = accp.tile([P, 1], BF16, tag="ones", name="ones")
        dume = accp.tile([P, 1], BF16, tag="dume", name="dume")
        colsb = accp.tile([P, 1024], F32, tag="colsb", name="colsb")
        red = [redp.tile([P, 512], F32, tag=f"red{t}", name=f"red{t}")
               for t in range(2)]
        nc.vector.memset(ones, 1.0)
        # pull the exp ACT_TABLE_LOAD into the DMA fill window
        nc.scalar.activation(dume, ones, AF.Exp, scale=1.0)

        # mini copy of the m=0 stationary columns so the very first matmuls
        # gate on a 32KB DMA instead of the full 256KB slab tile
        xa0 = xtp.tile([P, KC * P], F8, tag="xa0", name="xa0")
        # scalar HWDGE ring: lands in parallel with the sync ring's first DMA
        nc.scalar.dma_start(
            out=xa0, in_=xth[:, 0:P].rearrange("(k p) c -> p k c", k=KC))

        # plain DMAs of host-pretransposed keys; one 3D-AP DMA per tile
        # loads both k-halves (dest free block k*w..k*w+w <- xT rows
        # k*128..k*128+128, cols s..s+w); first sweep's moving tile first
        for t in (1, 0, 2, 3, 4):
            s, w = XT[t]
            nc.sync.dma_start(
                out=xts[t],
                in_=xth[:, s:s + w].rearrange("(k p) c -> p k c", k=KC))

        # 3D views [p, k, c] for DoubleRow matmuls (both k-tiles in one MM)
        xt3 = [xts[t].rearrange("p (k c) -> p k c", k=KC)
               for t in range(len(XT))]
        xa03 = xa0.rearrange("p (k c) -> p k c", k=KC)

        def mov(col):  # moving operand [128, 2, 512] for global column col
            t, off = _xt_tile(col)
            return xt3[t][:, :, off:off + 512]

        escs = {}
        pools = {0: escp0, 1: escp1, 2: escp2, 3: escp3}

        def red_mms(m, which):
            for i, (rc, off) in enumerate(RED):
                if rc not in which:
                    continue
                t, bp = i % 2, 32 * (i // 2)
                nc.tensor.matmul(
                    red[t][bp:bp + 1, :],
                    ones,
                    escs[(m, rc)][:, off:off + 512],
                    start=(m == 0), stop=(m == MT - 1),
                    skip_group_check=True)

        last = SWEEPS[-1][2]
        for cs, w, c in SWEEPS:
            for m in range(MT):
                ps = psp.tile([P, 1536], F32, tag="ps", name="ps")
                stat = xa03 if m == 0 else xt3[0][:, :, m * P:(m + 1) * P]
                for half in range(w // 512):
                    nc.tensor.matmul(
                        ps[:, half * 512:(half + 1) * 512],
                        stat,
                        mov(cs + half * 512),
                        start=True, stop=True,
                        perf_mode=mybir.MatmulPerfMode.DoubleRow)
                ridx = m * NCH + c
                if c == ACC_CH:
                    # b4's exp values feed nothing but the row sum: keep
                    # them in PSUM (faster ScalarE port, no SBUF tile)
                    nc.scalar.activation(
                        ps[:, w:2 * w], ps[:, :w], AF.Exp, scale=ESC,
                        accum_out=rs[:, ridx:ridx + 1])
                    continue
                esc = pools[c].tile([P, w], BF16, tag="esc", name="esc")
                escs[(m, c)] = esc
                if c == last and m >= MT - 4:
                    nc.scalar.activation(
                        esc, ps[:, :w], AF.Exp, scale=ESC,
                        accum_out=rs[:, ridx:ridx + 1])
                else:  # row sum on DVE, off the scalar engine
                    nc.scalar.activation(esc, ps[:, :w], AF.Exp, scale=ESC)
                    nc.vector.reduce_sum(rs[:, ridx:ridx + 1], esc,
                                         axis=mybir.AxisListType.X)
                # column-sum matmuls ride the last sweep: slices reading
                # the finished c0/c1 sweeps go with the current m; the two
                # slices reading this sweep's own exp output go one behind
                if c == last:
                    red_mms(m, (0, 1))
                    if m > 0:
                        red_mms(m - 1, (2,))
        red_mms(MT - 1, (2,))

        for t in range(2):
            nc.vector.tensor_copy(colsb[:, t * 512:(t + 1) * 512], red[t])
        nc.sync.dma_start(out=rs_out, in_=rs)
        # only partitions {0,32,64} carry column sums: ship 12KB, not 512KB
        nc.sync.dma_start(out=cols_out, in_=colsb[0:65:32, :])


def build_nc():
    nc = bacc.Bacc("TRN2", target_bir_lowering=False, debug=False,
                   num_devices=NCORES)
    xth = nc.dram_tensor("xt", [D, N2], F8, kind="ExternalInput").ap()
    rs_out = nc.dram_tensor("rs", [P, NCH * MT], F32,
                            kind="ExternalOutput").ap()
    cols_out = nc.dram_tensor("cols", [3, 1024], F32,
                              kind="ExternalOutput").ap()
    with tile.TileContext(nc) as tc:
        _build_kernel(tc, rs_out, cols_out, xth)
    nc.compile()
    return nc


_NC = None
LAST_RESULTS = None


def _make_in_maps(xq: np.ndarray):
    in_maps = []
    for c in range(NCORES):
        xr = np.roll(xq, -c * ROWS, axis=0)
        in_maps.append({"xt": np.ascontiguousarray(xr.T)})
    return in_maps


def kernel(representation: np.ndarray, **run_kwargs) -> np.ndarray:
    global _NC, LAST_RESULTS
    rep = np.asarray(representation, dtype=np.float32)
    assert rep.shape == (N2, D)

    # host prep: normalize (f32, matching torch CosineSimilarity eps),
    # scale by QS and quantize to fp8e4m3 for DoubleRow matmuls
    norms = np.maximum(np.sqrt((rep.astype(np.float64) ** 2).sum(axis=1)),
                       1e-8)
    xn = (rep / norms[:, None]).astype(np.float32)
    xq = (xn * QS).astype(ml_dtypes.float8_e4m3fn)

    if _NC is None:
        _NC = build_nc()
    res = run_bass_kernel_spmd(_NC, _make_in_maps(xq),
                               core_ids=list(range(NCORES)), **run_kwargs)
    LAST_RESULTS = res

    # combine row partials (rolled cols 0..5120) and column partials
    # (rolled cols 1024..4096, blocks b=1..3) into full row sums S
    S = np.zeros(N2, dtype=np.float64)
    for c, r in enumerate(res.results):
        rs = r["rs"].astype(np.float64).reshape(P, MT, NCH)  # [p, m, ch]
        own = rs.sum(axis=2).T.reshape(ROWS)                 # row m*128+p
        S[c * ROWS:(c + 1) * ROWS] += own
        cols = r["cols"].astype(np.float64)                  # [3, 1024]
        for i in range(6):
            colsum = cols[i // 2, (i % 2) * 512:(i % 2) * 512 + 512]
            g0 = (c * ROWS + 1024 + 512 * i) % N2
            S[g0:g0 + 512] += colsum

    # host tail: remove diagonal (as the device computed it, from fp8
    # inputs), add positive terms, final log/mean
    xb = xq.astype(np.float64) / QS
    ssb = (xb * xb).sum(axis=1)                  # device's sim[i,i]
    denom = S - np.exp(2.0 * ssb)
    xn64 = xn.astype(np.float64)
    pos = (xn64 * np.roll(xn64, -N, axis=0)).sum(axis=1)
    loss = (np.log(denom) - 2.0 * pos).mean()
    return np.asarray(np.float32(loss))


# revision 47
# speedup vs baseline: 1.0615x; 1.0084x over previous
"""NT-Xent (SimCLR) contrastive loss on 8 Trainium2 NeuronCores.

Math: with x = row-normalized representation [8192, 256], tau = 0.5,
  sim = x @ x.T
  loss = (1/8192) * sum_i [ ln(sum_{j != i} exp(2 sim[i,j])) - 2 sim[i, pos(i)] ]
where pos(i) = (i + 4096) mod 8192.

Split of work:
  Host (O(N*D), numpy): row-normalize, cast to bf16, per-core row-rolled
  and transposed copies, positive-pair dots, final combine/ln/mean.
  Device (O(N^2*D)): the similarity matrix and row/column sums of exp(2 sim).

Symmetry: sim is symmetric, so only 5/8 of it is computed. Each core gets
xT rolled so its 1024 slab rows sit at columns 0..1023 of xT, and computes
sim[0:1024, 0:5120] (column blocks b=0..4; b=4 is computed by both members
of a (c, c+4) pair). Row sums of exp(2 sim) cover blocks 0..4; the missing
blocks 5..7 are transposes of blocks 1..3 of three other cores, recovered
from COLUMN sums of exp over blocks 1..3, computed on-device with
ones-stationary matmuls accumulating in two PSUM banks across the 8
row-tiles. The host combines row + column partials into full row sums.

Device kernel (SPMD, identical program on all 8 cores):
  1. The host supplies xT [256, 8192] bf16 (normalized, rolled,
     transposed), so SBUF keys load with 5 plain chunk-aligned DMAs
     (both k-halves per chunk in one 3D-AP DMA). The slab tile and the
     first sweep's tile come first.
  2. Four chunk-sweeps over columns, widths {512,1536,1536,1536}, the
     cheap 512 sweep first (it depends on two small DMAs and its dense
     matmul stream warms the HAM clock gate). Per row-tile m: k-outer
     bf16 matmuls (N=512) into a [128,1536] PSUM tile (2 bufs, 6 banks),
     then one scalar Exp (scale=2) per chunk -> bf16 SBUF. The exp
     activation table is preloaded during the DMA fill.
  3. Row sums: DVE reduce_sum for the three 1536 sweeps (the DVE is
     otherwise idle), accum_out on the 512 sweep -> rs [128, 32].
  4. Column sums: six ones-stationary matmuls per m over exp'd
     [128,512] slices of blocks 1..3, accumulated over m in two PSUM
     banks at partitions {0,32,64}, emitted one m behind in the last
     sweep to keep the PE stream dense. DVE copies them out at the end.
"""

import numpy as np
import ml_dtypes

import concourse.bacc as bacc
import concourse.tile as tile
from concourse import mybir
from concourse.bass_utils import run_bass_kernel_spmd

N2 = 8192            # total rows (2N)
D = 256              # feature dim
NCORES = 8
ROWS = N2 // NCORES  # 1024 slab rows per core
N = N2 // 2          # positive-pair offset
P = 128              # SBUF partitions
KC = D // P          # 2 contraction chunks of 128
MT = ROWS // P       # 8 slab row-tiles
COLS = 5120          # columns computed per slab row-tile (blocks 0..4)
# xT SBUF tiles: (col start, width); tile 0 holds the slab columns
XT = [(0, 1024), (4608, 512), (1024, 512), (1536, 1536), (3072, 1536)]
# chunk sweeps in execution order: (col start, width, chunk id)
SWEEPS = [(4608, 512, 3), (0, 1536, 0), (1536, 1536, 1), (3072, 1536, 2)]
NCH = 4
ACC_CH = 3           # the only chunk using scalar accum_out (rest: DVE)
# column-sum slices covering rolled cols [1024, 4096) as (chunk, offset)
RED = [(0, 1024), (1, 0), (1, 512), (1, 1024), (2, 0), (2, 512)]

F32 = mybir.dt.float32
BF16 = mybir.dt.bfloat16
F8 = mybir.dt.float8e4
AF = mybir.ActivationFunctionType
QS = 16.0            # fp8 quantization scale for xT (values ~N(0, 1/16))
ESC = 2.0 / (QS * QS)  # exp scale: sim = psum / QS^2, arg = 2*sim


def _xt_tile(col):
    for t, (s, w) in enumerate(XT):
        if s <= col < s + w:
            return t, col - s
    raise ValueError(col)


def _build_kernel(tc: tile.TileContext, rs_out, cols_out, xth):
    nc = tc.nc
    with (
        tc.tile_pool(name="xt", bufs=1) as xtp,
        tc.tile_pool(name="acc", bufs=1) as accp,
        tc.tile_pool(name="esc0", bufs=MT) as escp0,
        tc.tile_pool(name="esc1", bufs=MT) as escp1,
        tc.tile_pool(name="esc2", bufs=MT) as escp2,
        tc.tile_pool(name="esc3", bufs=2) as escp3,
        tc.tile_pool(name="ps", bufs=2, space="PSUM") as psp,
        tc.tile_pool(name="red", bufs=1, space="PSUM") as redp,
    ):
        # each xT tile holds both k-halves: [:, k*w : k*w+w]
        xts = [xtp.tile([P, KC * w], F8, tag=f"xt{t}", name=f"xt{t}")
               for t, (s, w) in enumerate(XT)]
        rs = accp.tile([P, NCH * MT], F32, tag="rs", name="rs")
        ones = accp.tile([P, 1], BF16, tag="ones", name="ones")
        dume = accp.tile([P, 1], BF16, tag="dume", name="dume")
        colsb = accp.tile([P, 1024], F32, tag="colsb", name="colsb")
        red = [redp.tile([P, 512], F32, tag=f"red{t}", name=f"red{t}")
               for t in range(2)]
        nc.vector.memset(ones, 1.0)
        # pull the exp ACT_TABLE_LOAD into the DMA fill window
        nc.scalar.activation(dume, ones, AF.Exp, scale=1.0)

        # mini copy of the m=0 stationary columns so the very first matmuls
        # gate on a 32KB DMA instead of the full 256KB slab tile
        xa0 = xtp.tile([P, KC * P], F8, tag="xa0", name="xa0")
        # scalar HWDGE ring: lands in parallel with the sync ring's first DMA
        nc.scalar.dma_start(
            out=xa0, in_=xth[:, 0:P].rearrange("(k p) c -> p k c", k=KC))

        # plain DMAs of host-pretransposed keys; one 3D-AP DMA per tile
        # loads both k-halves (dest free block k*w..k*w+w <- xT rows
        # k*128..k*128+128, cols s..s+w); first sweep's moving tile first
        for t in (1, 0, 2, 3, 4):
            s, w = XT[t]
            nc.sync.dma_start(
                out=xts[t],
                in_=xth[:, s:s + w].rearrange("(k p) c -> p k c", k=KC))

        # 3D views [p, k, c] for DoubleRow matmuls (both k-tiles in one MM)
        xt3 = [xts[t].rearrange("p (k c) -> p k c", k=KC)
               for t in range(len(XT))]
        xa03 = xa0.rearrange("p (k c) -> p k c", k=KC)

        def mov(col):  # moving operand [128, 2, 512] for global column col
            t, off = _xt_tile(col)
            return xt3[t][:, :, off:off + 512]

        escs = {}
        pools = {0: escp0, 1: escp1, 2: escp2, 3: escp3}

        def red_mms(m, which):
            # slices 0-2 -> red[0] (finishes at the c0/c1-fed matmuls of
            # m=7, so its copy overlaps the final chunk-2 matmuls),
            # slices 3-5 -> red[1]
            for i, (rc, off) in enumerate(RED):
                if rc not in which:
                    continue
                t, bp = i // 3, 32 * (i % 3)
                nc.tensor.matmul(
                    red[t][bp:bp + 1, :],
                    ones,
                    escs[(m, rc)][:, off:off + 512],
                    start=(m == 0), stop=(m == MT - 1),
                    skip_group_check=True)

        last = SWEEPS[-1][2]
        for cs, w, c in SWEEPS:
            for m in range(MT):
                ps = psp.tile([P, 1536], F32, tag="ps", name="ps")
                stat = xa03 if m == 0 else xt3[0][:, :, m * P:(m + 1) * P]
                for half in range(w // 512):
                    nc.tensor.matmul(
                        ps[:, half * 512:(half + 1) * 512],
                        stat,
                        mov(cs + half * 512),
                        start=True, stop=True,
                        perf_mode=mybir.MatmulPerfMode.DoubleRow)
                ridx = m * NCH + c
                if c == ACC_CH:
                    # b4's exp values feed nothing but the row sum: keep
                    # them in PSUM (faster ScalarE port, no SBUF tile)
                    nc.scalar.activation(
                        ps[:, w:2 * w], ps[:, :w], AF.Exp, scale=ESC,
                        accum_out=rs[:, ridx:ridx + 1])
                    continue
                esc = pools[c].tile([P, w], BF16, tag="esc", name="esc")
                escs[(m, c)] = esc
                if c == last and m >= MT - 4:
                    nc.scalar.activation(
                        esc, ps[:, :w], AF.Exp, scale=ESC,
                        accum_out=rs[:, ridx:ridx + 1])
                else:  # row sum on DVE, off the scalar engine
                    nc.scalar.activation(esc, ps[:, :w], AF.Exp, scale=ESC)
                    nc.vector.reduce_sum(rs[:, ridx:ridx + 1], esc,
                                         axis=mybir.AxisListType.X)
                # column-sum matmuls ride the last sweep: slices reading
                # the finished c0/c1 sweeps go with the current m; the two
                # slices reading this sweep's own exp output go one behind
                if c == last:
                    red_mms(m, (0, 1))
                    if m > 0:
                        red_mms(m - 1, (2,))
        red_mms(MT - 1, (2,))

        for t in range(2):
            nc.vector.tensor_copy(colsb[:, t * 512:(t + 1) * 512], red[t])
        nc.sync.dma_start(out=rs_out, in_=rs)
        # only partitions {0,32,64} carry column sums: ship 12KB, not 512KB
        nc.sync.dma_start(out=cols_out, in_=colsb[0:65:32, :])


def build_nc():
    nc = bacc.Bacc("TRN2", target_bir_lowering=False, debug=False,
                   num_devices=NCORES)
    xth = nc.dram_tensor("xt", [D, N2], F8, kind="ExternalInput").ap()
    rs_out = nc.dram_tensor("rs", [P, NCH * MT], F32,
                            kind="ExternalOutput").ap()
    cols_out = nc.dram_tensor("cols", [3, 1024], F32,
                              kind="ExternalOutput").ap()
    with tile.TileContext(nc) as tc:
        _build_kernel(tc, rs_out, cols_out, xth)
    nc.compile()
    return nc


_NC = None
LAST_RESULTS = None


def _make_in_maps(xq: np.ndarray):
    in_maps = []
    for c in range(NCORES):
        xr = np.roll(xq, -c * ROWS, axis=0)
        in_maps.append({"xt": np.ascontiguousarray(xr.T)})
    return in_maps


def kernel(representation: np.ndarray, **run_kwargs) -> np.ndarray:
    global _NC, LAST_RESULTS
    rep = np.asarray(representation, dtype=np.float32)
    assert rep.shape == (N2, D)

    # host prep: normalize (f32, matching torch CosineSimilarity eps),
    # scale by QS and quantize to fp8e4m3 for DoubleRow matmuls
    norms = np.maximum(np.sqrt((rep.astype(np.float64) ** 2).sum(axis=1)),
                       1e-8)
    xn = (rep / norms[:, None]).astype(np.float32)
    xq = (xn * QS).astype(ml_dtypes.float8_e4m3fn)

    if _NC is None:
        _NC = build_nc()
    res = run_bass_kernel_spmd(_NC, _make_in_maps(xq),
                               core_ids=list(range(NCORES)), **run_kwargs)
    LAST_RESULTS = res

    # combine row partials (rolled cols 0..5120) and column partials
    # (rolled cols 1024..4096, blocks b=1..3) into full row sums S
    S = np.zeros(N2, dtype=np.float64)
    for c, r in enumerate(res.results):
        rs = r["rs"].astype(np.float64).reshape(P, MT, NCH)  # [p, m, ch]
        own = rs.sum(axis=2).T.reshape(ROWS)                 # row m*128+p
        S[c * ROWS:(c + 1) * ROWS] += own
        cols = r["cols"].astype(np.float64)                  # [3, 1024]
        for i in range(6):
            colsum = cols[i % 3, (i // 3) * 512:(i // 3) * 512 + 512]
            g0 = (c * ROWS + 1024 + 512 * i) % N2
            S[g0:g0 + 512] += colsum

    # host tail: remove diagonal (as the device computed it, from fp8
    # inputs), add positive terms, final log/mean
    xb = xq.astype(np.float64) / QS
    ssb = (xb * xb).sum(axis=1)                  # device's sim[i,i]
    denom = S - np.exp(2.0 * ssb)
    xn64 = xn.astype(np.float64)
    pos = (xn64 * np.roll(xn64, -N, axis=0)).sum(axis=1)
    loss = (np.log(denom) - 2.0 * pos).mean()
    return np.asarray(np.float32(loss))
